# revision 21
# baseline (speedup 1.0000x reference)
"""DeepseekV3 MoE "calibrate-all-experts" kernel for 8 Trainium2 NeuronCores.

Only the top-8 experts per token contribute to the output (the dense [T,E]
combine weight is 0 elsewhere), so instead of running all 32 experts over
all 2048 tokens, the host computes the router, gathers each expert's
selected tokens into a padded capacity buffer, and the device runs dense
matmuls only on those (~1/4 the flops).

Sharding: each core owns 4 expert "slots". Experts are ranked by token
count; slot j holds ranks [8j, 8j+8) (one per core) and has a single
data-independent capacity cap_j = roundup(max count in slot, 32), so the
SPMD instruction stream is identical across cores while the gathered data
differs. The shared expert is sharded over its intermediate dim (exact,
since the SwiGLU nonlinearity is elementwise in IS); each core emits a
partial [T,H] that the host sums. The shared phase runs FIRST (its first
matmul needs only ~2MB of input) and routed slots run after, with weight
tiles split in halves so the next slot's weights prefetch during the
current slot's compute.

All device tensors are laid out partition-major on the host ([P, k, free])
so each DMA line is one long descriptor per partition.

On-device math: bf16 matmuls with fp32 PSUM accumulation; silu via the
scalar engine; the per-(token,expert) combine weight is folded into the
activations before the down projection, so padded slots contribute exact
zeros. The host scatter-adds the per-expert outputs back to token order
(vectorized per top-k column, where token indices are unique).
"""
import sys

if '/opt/trn_rl_repo' not in sys.path:
    sys.path.insert(0, '/opt/trn_rl_repo')

import numpy as np
import ml_dtypes

import concourse.bass as bass
import concourse.mybir as mybir
import concourse.tile as tile
from concourse import bacc
from concourse.bass import ds, ts
from concourse.bass_utils import run_bass_kernel_spmd

F32 = mybir.dt.float32
BF16 = mybir.dt.bfloat16
P = 128

FULL = dict(T=2048, H=2048, E=32, I=1024, IS=2048, n_cores=8)
TOP_K = 8
ROUTED_SCALING = 2.5
SC = 256          # shared-expert token chunk
CH = 512          # routed token chunk (phase-1 moving dim)


def build_moe_nc(T, H, E, I, IS, n_cores, caps):
    E_LOC = len(caps)
    IS_LOC = IS // n_cores
    KH = H // P           # contraction tiles over H
    KI = I // P           # expert intermediate tiles
    KIS = IS_LOC // P     # shared intermediate tiles (per core)
    NHC = H // 512        # output h chunks (shared down)
    NSC = T // SC
    IH = I // 2           # weight half width (gate/up)
    HH = H // 2           # weight half width (down)
    offs = np.concatenate([[0], np.cumsum(caps)]).astype(int)
    SLOT_TOT = int(offs[-1])
    CAPMAX = int(max(caps))
    assert H % P == 0 and I % P == 0 and IS_LOC % P == 0 and T % SC == 0

    nc = bacc.Bacc("TRN2", target_bir_lowering=False, debug=False,
                   num_devices=n_cores)

    # all inputs partition-major: one long DMA line per partition
    xgs = [nc.dram_tensor(f"xg{j}", [P, KH, int(caps[j])], BF16,
                          kind="ExternalInput") for j in range(E_LOC)]
    wv = nc.dram_tensor("wv", [SLOT_TOT], F32, kind="ExternalInput")
    wg = nc.dram_tensor("wg", [E_LOC, 2, P, KH, IH], BF16,
                        kind="ExternalInput")
    wu = nc.dram_tensor("wu", [E_LOC, 2, P, KH, IH], BF16,
                        kind="ExternalInput")
    wd = nc.dram_tensor("wd", [E_LOC, 2, P, KI, HH], BF16,
                        kind="ExternalInput")
    xs = nc.dram_tensor("xs", [NSC, P, KH, SC], BF16, kind="ExternalInput")
    wsg = nc.dram_tensor("wsg", [KIS, P, KH, P], BF16, kind="ExternalInput")
    wsu = nc.dram_tensor("wsu", [KIS, P, KH, P], BF16, kind="ExternalInput")
    wsd = nc.dram_tensor("wsd", [P, KIS, H], BF16, kind="ExternalInput")
    eo = nc.dram_tensor("eo", [H, SLOT_TOT], BF16, kind="ExternalOutput")
    sh = nc.dram_tensor("sh", [T, H], BF16, kind="ExternalOutput")

    with tile.TileContext(nc) as tc:
        with (
            tc.tile_pool(name="cpool", bufs=1) as cpool,
            tc.tile_pool(name="xgpool", bufs=2) as xgpool,
            tc.tile_pool(name="wpool", bufs=4) as wpool,
            tc.tile_pool(name="sgpool", bufs=18) as sgpool,
            tc.tile_pool(name="apool", bufs=20) as apool,
            tc.tile_pool(name="ashpool", bufs=4) as ashpool,
            tc.tile_pool(name="xspool", bufs=2) as xspool,
            tc.tile_pool(name="wshpool", bufs=3) as wshpool,
            tc.tile_pool(name="opool", bufs=4) as opool,
            tc.tile_pool(name="pgp", bufs=4, space="PSUM") as pgp,
            tc.tile_pool(name="pop", bufs=4, space="PSUM") as pop,
        ):
            # ---------------- shared expert (IS sharded), runs first -----
            # wsg/wsu split into i2-halves so the first matmul only waits
            # for ~1MB of input.
            assert KIS == 2
            wsh_h = {}
            for nm, src in (("g", wsg), ("u", wsu)):
                for i2 in range(KIS):
                    t = wshpool.tile([P, KH, P], BF16, tag="wsh_h",
                                     bufs=4, name=f"ws{nm}{i2}")
                    nc.sync.dma_start(t[:], src.ap()[i2])
                    wsh_h[(nm, i2)] = t
            wsd_sb = wshpool.tile([P, KIS, H], BF16, tag="wsh_d", bufs=1)
            nc.sync.dma_start(wsd_sb[:], wsd.ap())

            # prefetch state for the routed slots (emitted mid-shared);
            # big prefetches are gated on shared-chunk progress via dummy
            # pool tiles so they can't starve the critical early loads.
            wtiles = {}
            gate_tiles = []

            def emit_gates(pool, n, tag, dep, gsh):
                for g in range(n):
                    dmy = pool.tile([P, 4], BF16, tag=tag, name=f"gate{g}")
                    nc.vector.tensor_copy(dmy[:1, :1], dep[:1, :1])
                    gate_tiles.append(dmy)

            def load_w(src, j, half):
                kd, fw = (KI, HH) if src is wd else (KH, IH)
                t = wpool.tile([P, kd, fw], BF16, tag="w",
                               name=f"w_{j}_{half}")
                nc.sync.dma_start(t[:], src.ap()[j][half])
                return t

            for tc4 in range(NSC):
                xs_sb = xspool.tile([P, KH, SC], BF16, tag="xs")
                nc.sync.dma_start(xs_sb[:], xs.ap()[tc4])
                ash = []
                for i2 in range(KIS):
                    pg = pgp.tile([P, 512], F32, tag="pg")
                    for k in range(KH):
                        nc.tensor.matmul(
                            pg[:, :SC], wsh_h[("g", i2)][:, k, :],
                            xs_sb[:, k, :],
                            start=(k == 0), stop=(k == KH - 1))
                    sg = sgpool.tile([P, 512], BF16, tag="sg")
                    nc.scalar.activation(
                        sg[:, :SC], pg[:, :SC],
                        mybir.ActivationFunctionType.Silu)
                    pu = pgp.tile([P, 512], F32, tag="pg")
                    for k in range(KH):
                        nc.tensor.matmul(
                            pu[:, :SC], wsh_h[("u", i2)][:, k, :],
                            xs_sb[:, k, :],
                            start=(k == 0), stop=(k == KH - 1))
                    a = ashpool.tile([P, 512], BF16, tag="ash")
                    nc.vector.tensor_mul(a[:, :SC], sg[:, :SC], pu[:, :SC])
                    ash.append(a)
                for t in range(SC // P):
                    for hc in range(NHC):
                        po = pop.tile([P, 512], F32, tag="po")
                        for i2 in range(KIS):
                            nc.tensor.matmul(
                                po[:], ash[i2][:, ts(t, P)],
                                wsd_sb[:, i2, ds(hc * 512, 512)],
                                start=(i2 == 0), stop=(i2 == KIS - 1))
                        ost = opool.tile([P, 512], BF16, tag="ost")
                        nc.vector.tensor_copy(ost[:], po[:])
                        nc.sync.dma_start(
                            sh.ap()[ds(tc4 * SC + t * P, P),
                                    ds(hc * 512, 512)],
                            ost[:])
                # gate dummies: slot-0 prefetch DMAs inherit these pool
                # slots, so they can't start before these chunks finish
                if tc4 == 0:
                    emit_gates(wpool, 2, "w", ash[0], [P, KI, HH])
                    emit_gates(xgpool, 2, "xg", ash[0], [P, KH, CAPMAX])
                    emit_gates(cpool, 1, "wvb", ash[0], [P, 4])
                if tc4 == 1:
                    emit_gates(wpool, 2, "w", ash[0], [P, KI, HH])
                # prefetch slot 0's inputs while the shared phase computes
                if tc4 == 2:
                    xg_sb0 = xgpool.tile([P, KH, CAPMAX], BF16, tag="xg")
                    nc.sync.dma_start(xg_sb0[:, :, :int(caps[0])],
                                      xgs[0].ap())
                    wtiles[("g", 0, 0)] = load_w(wg, 0, 0)
                    wtiles[("g", 0, 1)] = load_w(wg, 0, 1)
                    wvb = cpool.tile([P, SLOT_TOT], F32, tag="wvb")
                    nc.sync.dma_start(wvb[:], wv.ap()[ds(0, SLOT_TOT)]
                                      .partition_broadcast(P))
                if tc4 == 5:
                    wtiles[("u", 0, 0)] = load_w(wu, 0, 0)
                    wtiles[("u", 0, 1)] = load_w(wu, 0, 1)

            # ---------------- routed experts (E_LOC slots) ----------------
            xg_tiles = {0: xg_sb0}
            for j in range(E_LOC):
                cap = int(caps[j])
                off = int(offs[j])
                xg_sb = xg_tiles.pop(j)
                wgA = wtiles.pop(("g", j, 0))
                wgB = wtiles.pop(("g", j, 1))

                # phase 1a: gate projection, sg = silu(g) stored bf16
                sgs = {}
                for c0 in range(0, cap, CH):
                    cl = min(CH, cap - c0)
                    for i in range(KI):
                        wgh = wgA if i < KI // 2 else wgB
                        il = i % (KI // 2)
                        pg = pgp.tile([P, 512], F32, tag="pg")
                        for k in range(KH):
                            nc.tensor.matmul(
                                pg[:, :cl], wgh[:, k, ts(il, P)],
                                xg_sb[:, k, ds(c0, cl)],
                                start=(k == 0), stop=(k == KH - 1))
                        sg = sgpool.tile([P, 512], BF16, tag="sg")
                        nc.scalar.activation(
                            sg[:, :cl], pg[:, :cl],
                            mybir.ActivationFunctionType.Silu)
                        sgs[(i, c0)] = sg

                # prefetch down weights (halves) for this slot
                wdA = load_w(wd, j, 0)
                wdB = load_w(wd, j, 1)
                wuA = wtiles.pop(("u", j, 0))
                wuB = wtiles.pop(("u", j, 1))

                # phase 1b: up projection, act = sg * u * combine_weight
                acts = {}
                for c0 in range(0, cap, CH):
                    cl = min(CH, cap - c0)
                    for i in range(KI):
                        wuh = wuA if i < KI // 2 else wuB
                        il = i % (KI // 2)
                        pu = pgp.tile([P, 512], F32, tag="pg")
                        for k in range(KH):
                            nc.tensor.matmul(
                                pu[:, :cl], wuh[:, k, ts(il, P)],
                                xg_sb[:, k, ds(c0, cl)],
                                start=(k == 0), stop=(k == KH - 1))
                        a = apool.tile([P, 512], BF16, tag="act")
                        nc.vector.tensor_mul(a[:, :cl], sgs[(i, c0)][:, :cl],
                                             pu[:, :cl])
                        nc.vector.tensor_mul(a[:, :cl], a[:, :cl],
                                             wvb[:, ds(off + c0, cl)])
                        acts[(i, c0)] = a

                # prefetch next slot's x and gate weights
                if j + 1 < E_LOC:
                    nxt = xgpool.tile([P, KH, CAPMAX], BF16, tag="xg",
                                      name=f"xg_sb{j + 1}")
                    nc.sync.dma_start(nxt[:, :, :int(caps[j + 1])],
                                      xgs[j + 1].ap())
                    xg_tiles[j + 1] = nxt
                    wtiles[("g", j + 1, 0)] = load_w(wg, j + 1, 0)
                    wtiles[("g", j + 1, 1)] = load_w(wg, j + 1, 1)

                # phase 2: down projection (tokens moving), h-slice major
                for hs in range(KH):
                    wdh = wdA if hs < KH // 2 else wdB
                    hl = (hs % (KH // 2)) * P
                    for c0 in range(0, cap, CH):
                        cl = min(CH, cap - c0)
                        po = pop.tile([P, 512], F32, tag="po")
                        for i in range(KI):
                            nc.tensor.matmul(
                                po[:, :cl], wdh[:, i, ds(hl, P)],
                                acts[(i, c0)][:, :cl],
                                start=(i == 0), stop=(i == KI - 1))
                        ost = opool.tile([P, 512], BF16, tag="ost")
                        nc.vector.tensor_copy(ost[:, :cl], po[:, :cl])
                        nc.sync.dma_start(
                            eo.ap()[ds(hs * P, P), ds(off + c0, cl)],
                            ost[:, :cl])
                    # prefetch next slot's up weights mid-down
                    if hs == KH // 2 and j + 1 < E_LOC:
                        wtiles[("u", j + 1, 0)] = load_w(wu, j + 1, 0)
                        wtiles[("u", j + 1, 1)] = load_w(wu, j + 1, 1)

    nc.compile()
    return nc


_NC_CACHE = {}


def _get_nc(caps):
    key = tuple(caps)
    if key not in _NC_CACHE:
        _NC_CACHE[key] = build_moe_nc(**FULL, caps=list(caps))
    return _NC_CACHE[key]


def _pmaj(a, P=128):
    """[K*P, F...] -> [P, K, F] partition-major, contiguous."""
    K = a.shape[0] // P
    F = int(np.prod(a.shape[1:]))
    return np.ascontiguousarray(
        a.reshape(K, P, F).transpose(1, 0, 2))


def _wsH(a, P=128):
    """[KH*P, KIS*P] -> [KIS, P, KH, P] i2-half-major, contiguous."""
    KH = a.shape[0] // P
    KIS = a.shape[1] // P
    return np.ascontiguousarray(
        a.reshape(KH, P, KIS, P).transpose(2, 1, 0, 3))


def prepare(hidden_states, gate_weight, w_gate, w_up, w_down,
            ws_gate, ws_up, ws_down):
    """Host routing + gather. Returns (caps, in_maps, meta)."""
    B, S, H = hidden_states.shape
    T = B * S
    E = gate_weight.shape[0]
    IS = ws_gate.shape[1]
    n_cores = FULL["n_cores"]
    E_LOC = E // n_cores
    IS_LOC = IS // n_cores
    KH = H // P
    bf16 = ml_dtypes.bfloat16

    x32 = np.asarray(hidden_states, np.float32).reshape(T, H)
    logits = x32 @ np.asarray(gate_weight, np.float32).T
    scores = 1.0 / (1.0 + np.exp(-logits, dtype=np.float32))
    part = np.argpartition(-scores, TOP_K - 1, axis=1)[:, :TOP_K]
    w8 = np.take_along_axis(scores, part, 1)
    wts = (w8 / (w8.sum(1, keepdims=True) + 1e-20)
           * ROUTED_SCALING).astype(np.float32)

    flat_e = part.ravel()
    flat_t = np.repeat(np.arange(T, dtype=np.int64), TOP_K)
    flat_k = np.tile(np.arange(TOP_K, dtype=np.int64), T)
    flat_w = wts.ravel()
    ordx = np.argsort(flat_e, kind="stable")
    counts = np.bincount(flat_e, minlength=E)
    starts = np.concatenate([[0], np.cumsum(counts)]).astype(np.int64)
    rank = np.argsort(-counts, kind="stable")

    caps = []
    assign = np.empty((n_cores, E_LOC), dtype=np.int64)
    for j in range(E_LOC):
        grp = rank[j * n_cores:(j + 1) * n_cores]
        caps.append(max(32, int(-(-int(counts[grp].max()) // 8) * 8)))
        assign[:, j] = grp
    offs = np.concatenate([[0], np.cumsum(caps)]).astype(np.int64)
    SLOT_TOT = int(offs[-1])

    xb = x32.astype(bf16)                      # [T, H]
    xb_aug = np.vstack([xb, np.zeros((1, H), bf16)])   # row T = zero pad
    wgb = np.asarray(w_gate, np.float32).astype(bf16)
    wub = np.asarray(w_up, np.float32).astype(bf16)
    wdb = np.asarray(w_down, np.float32).astype(bf16)

    NSC = T // SC
    xsP = np.ascontiguousarray(
        xb.reshape(NSC, SC, KH, P).transpose(0, 3, 2, 1))

    def wP(wb, loc):  # [n, D*P, F] -> [n, 2, P, D, F/2] half-major
        a = wb[loc]
        n, D, F = a.shape[0], a.shape[1] // P, a.shape[2]
        return np.ascontiguousarray(
            a.reshape(n, D, P, 2, F // 2).transpose(0, 3, 2, 1, 4))

    in_maps = []
    rows_l, toks_l, ks_l = [], [], []
    for c in range(n_cores):
        wv_c = np.zeros(SLOT_TOT, dtype=np.float32)
        im = {}
        for j in range(E_LOC):
            e = int(assign[c, j])
            cnt = int(counts[e])
            cap = caps[j]
            sel = ordx[starts[e]:starts[e] + cnt]
            toks = flat_t[sel]
            ptoks = np.full(cap, T, dtype=np.int64)
            ptoks[:cnt] = toks
            blk = xb_aug[ptoks]                      # [cap, H]
            im[f"xg{j}"] = np.ascontiguousarray(
                blk.reshape(cap, KH, P).transpose(2, 1, 0))
            wv_c[offs[j]:offs[j] + cnt] = flat_w[sel]
            rows_l.append(c * SLOT_TOT + offs[j]
                          + np.arange(cnt, dtype=np.int64))
            toks_l.append(toks)
            ks_l.append(flat_k[sel])
        loc = assign[c]
        im.update({
            "wv": wv_c,
            "wg": wP(wgb, loc),
            "wu": wP(wub, loc),
            "wd": wP(wdb, loc),
            "xs": xsP,
            "wsg": _wsH(np.ascontiguousarray(
                ws_gate[:, c * IS_LOC:(c + 1) * IS_LOC]).astype(bf16)),
            "wsu": _wsH(np.ascontiguousarray(
                ws_up[:, c * IS_LOC:(c + 1) * IS_LOC]).astype(bf16)),
            "wsd": _pmaj(np.ascontiguousarray(
                ws_down[c * IS_LOC:(c + 1) * IS_LOC, :]).astype(bf16)),
        })
        in_maps.append(im)

    meta = dict(
        B=B, S=S, T=T, H=H, n_cores=n_cores,
        rows=np.concatenate(rows_l),
        toks=np.concatenate(toks_l),
        ks=np.concatenate(ks_l),
    )
    return caps, in_maps, meta


def finish(results, meta):
    """Sum shared partials and scatter-add routed expert outputs."""
    T, H = meta["T"], meta["H"]
    out = np.zeros((T, H), np.float32)
    for c in range(meta["n_cores"]):
        out += np.asarray(results[c]["sh"], dtype=np.float32)
    EO = np.concatenate(
        [np.ascontiguousarray(np.asarray(results[c]["eo"]).T)
         for c in range(meta["n_cores"])],
        axis=0).astype(np.float32)
    rows, toks, ks = meta["rows"], meta["toks"], meta["ks"]
    for k in range(TOP_K):
        m = ks == k
        out[toks[m]] += EO[rows[m]]
    return out.reshape(meta["B"], meta["S"], H)


def kernel(hidden_states, gate_weight, w_gate, w_up, w_down,
           ws_gate, ws_up, ws_down):
    caps, in_maps, meta = prepare(
        hidden_states, gate_weight, w_gate, w_up, w_down,
        ws_gate, ws_up, ws_down)
    nc = _get_nc(caps)
    res = run_bass_kernel_spmd(nc, in_maps,
                               core_ids=list(range(FULL["n_cores"])))
    out = finish(res.results, meta)
    return np.ascontiguousarray(
        out.astype(np.asarray(hidden_states).dtype))
